# revision 18
# baseline (speedup 1.0000x reference)
"""AttentionPooling kernel for Trainium2 (8 NeuronCores, SPMD, no collectives).

reference math:
    scores = tanh(x @ W1 + b1) @ W2 + b2        # [N, 1]
    attn   = softmax(scores, axis=0)            # global over all N rows
    pooled = segment_sum(x * attn, batch, 1024) # [1024, 256]

Strategy (v2, bf16):
  - batch is sorted, so shard ROWS at graph boundaries: core c gets all rows
    with batch in [128c, 128(c+1)).  Each core owns exactly 128 output graphs
    -> no cross-core reduction for pooled.
  - b2 cancels in softmax (constant shift) -> dropped.  b1 folds into the
    tanh activation bias (per-partition AP) when nonzero.
  - softmax normalizer: each core returns unnormalized A_g = sum_i e_i x_i and
    per-row e values; host divides by the global Z (exact).
  - all heavy tensors are bf16: host pre-packs BOTH layouts of x so the
    kernel never transposes on-chip:
      xrow[p, c, t*256+d]    = x[c*2048 + t*128 + p, d]      (pooling matmul)
      xcol[dl, c, dc*2048+i] = x[c*2048 + i, dc*128 + dl]    (MLP matmul)
  - per 4-tile group (512 rows):
      hT[j, i]  = sum_d W1[d, j] xT[d, i]     4 matmuls, 512-col bf16 streams
      thT       = tanh(hT (+ b1))             one ACT call, PSUM->SBUF bf16
      s[i]      = sum_j thT[j, i] W2[j]       8 tiny matmuls (thT stationary)
      e         = exp(s)                      one ACT call -> evec SBUF
      M[i, g]   = (iota == brel) * e          ONE fused DVE tensor_scalar
      acc[g, d] += M^T @ x_tile               bf16 matmul, PSUM-resident
"""

import numpy as np
from contextlib import ExitStack

import ml_dtypes
import concourse.bass as bass
import concourse.bacc as bacc
import concourse.mybir as mybir
import concourse.tile as tile
from concourse.bass_utils import run_bass_kernel_spmd

F32 = mybir.dt.float32
BF16 = mybir.dt.bfloat16
I32 = mybir.dt.int32

NUM_GRAPHS = 1024
NC = 8
GPC = NUM_GRAPHS // NC  # graphs per core = 128
P = 128
D = 256
CH = 16  # tiles per DMA chunk (2048 rows -> 1 MiB per layout)
GRP = 4  # tiles per compute group (512 rows)


def build_program(R: int, T: int, with_b1: bool) -> bass.Bass:
    assert T % CH == 0 and R == T * P
    C = T // CH  # DMA chunks

    nc = bacc.Bacc("TRN2", target_bir_lowering=False, debug=False)
    xrow = nc.declare_dram_parameter("xrow", [P, C, CH * D], BF16, isOutput=False)
    xcol = nc.declare_dram_parameter("xcol", [P, C, 2 * CH * P], BF16, isOutput=False)
    brel = nc.declare_dram_parameter("brel", [P, T], F32, isOutput=False)
    w1 = nc.declare_dram_parameter("w1", [P, 2, D], BF16, isOutput=False)
    w2 = nc.declare_dram_parameter("w2", [P, 2, 2], BF16, isOutput=False)
    if with_b1:
        b1d = nc.declare_dram_parameter("b1d", [P, 2], F32, isOutput=False)
    pooled = nc.declare_dram_parameter("pooled", [P, D], F32, isOutput=True)
    evec_out = nc.declare_dram_parameter("evec_out", [P, T], F32, isOutput=True)

    with ExitStack() as ctx:
        tc = ctx.enter_context(tile.TileContext(nc))
        const = ctx.enter_context(tc.tile_pool(name="const", bufs=1))
        xrp = ctx.enter_context(tc.tile_pool(name="xr", bufs=4))
        xcp = ctx.enter_context(tc.tile_pool(name="xc", bufs=4))
        htpp = ctx.enter_context(tc.tile_pool(name="htp", bufs=2, space="PSUM"))
        thp = ctx.enter_context(tc.tile_pool(name="th", bufs=2))
        spp = ctx.enter_context(tc.tile_pool(name="sp", bufs=2, space="PSUM"))
        mpl = ctx.enter_context(tc.tile_pool(name="m", bufs=20))
        accp = ctx.enter_context(tc.tile_pool(name="acc", bufs=1, space="PSUM"))
        accp2 = ctx.enter_context(tc.tile_pool(name="acc2", bufs=1, space="PSUM"))
        outp = ctx.enter_context(tc.tile_pool(name="out", bufs=1))

        # ---- constants ----
        iota_i = const.tile([P, P], I32)
        nc.gpsimd.iota(iota_i[:], pattern=[[1, P]], base=0, channel_multiplier=0)
        iota_f = const.tile([P, P], F32)
        nc.vector.tensor_copy(iota_f[:], iota_i[:])

        w1sb = const.tile([P, 2, D], BF16, tag="w1sb")  # [d_lo, dc, j]
        nc.sync.dma_start(w1sb[:], w1[:])
        w2sb = const.tile([P, 2, 2], BF16, tag="w2sb")  # [j_lo, jc, dup]
        nc.sync.dma_start(w2sb[:], w2[:])
        brelsb = const.tile([P, T], F32)
        nc.sync.dma_start(brelsb[:], brel[:])
        if with_b1:
            b1sb = const.tile([P, 2], F32, tag="b1sb")  # [j_lo, jc]
            nc.sync.dma_start(b1sb[:], b1d[:])

        evec = const.tile([P, T], F32, tag="evec")  # exp(s) per row
        # two PSUM accumulators in separate banks, alternating per tile so
        # consecutive pooling matmuls never WAW the same bank (drain overlap)
        accA = accp.tile([P, 512], F32, tag="accA")
        accB = accp2.tile([P, 512], F32, tag="accB")
        accs = [accA, accB]

        Tanh = mybir.ActivationFunctionType.Tanh
        Exp = mybir.ActivationFunctionType.Exp

        NG = T // GRP  # total compute groups

        def tanh_act(gt0, htp):
            """ACT tanh PSUM->SBUF."""
            th = thp.tile([P, 2, GRP * P], BF16)
            if with_b1:
                for jc in range(2):
                    nc.scalar.activation(
                        th[:, jc, :], htp[:, jc, :], Tanh,
                        bias=b1sb[:, jc : jc + 1],
                    )
            else:
                nc.scalar.activation(th[:], htp[:], Tanh)
            return th

        def masks(gt0):
            """DVE one-hot*e masks for one group."""
            ms = []
            for tt in range(GRP):
                gt = gt0 + tt
                m = mpl.tile([P, P], BF16)
                nc.vector.tensor_scalar(
                    m[:],
                    iota_f[:],
                    brelsb[:, gt : gt + 1],
                    evec[:, gt : gt + 1],
                    op0=mybir.AluOpType.is_equal,
                    op1=mybir.AluOpType.mult,
                )
                ms.append(m)
            return ms

        def scores_acc(sc, ac):
            """Interleave tiny score matmuls with pooling matmuls on PE."""
            sp = None
            if sc is not None:
                gt0_s, th = sc
                sp = spp.tile([P, GRP, 2], F32)
            for tt in range(GRP):
                if ac is not None:
                    gt0_a, xr, g, ms = ac
                    gt = gt0_a + tt
                    nc.tensor.matmul(
                        accs[gt % 2][:, 0:D],
                        lhsT=ms[tt][:],
                        rhs=xr[:, g * GRP + tt, :],
                        start=(gt < 2),
                        stop=(gt >= T - 2),
                        skip_group_check=True,
                    )
                if sc is not None:
                    for jc in range(2):
                        nc.tensor.matmul(
                            sp[:, tt, :],
                            lhsT=th[:, jc, tt * P : (tt + 1) * P],
                            rhs=w2sb[:, jc, :],
                            start=(jc == 0),
                            stop=(jc == 1),
                            skip_group_check=True,
                        )
            if sc is not None:
                nc.scalar.activation(evec[:, gt0_s : gt0_s + GRP], sp[:, :, 0], Exp)

        # Software pipeline, iteration k issues:
        #   masks(k-2) on DVE first (evec ready since k-1 -> DVE runs ahead)
        #   h(k), acc(k-3), scores(k-1) on PE
        #   tanh(k-1)/exp(k-1) on ACT
        # so every PE matmul's operands are >= 1 full iteration old.
        pend = []  # groups awaiting tanh/scores/exp
        pend_m = []  # groups awaiting mask build (evec ready)
        pend_acc = []  # groups with masks built, awaiting pool matmul
        xr = None

        def step(newgrp):
            # pop the acc group FIRST so fresh masks get a full iteration of
            # DVE lead time before PE consumes them
            ac = pend_acc.pop(0) if pend_acc else None
            if pend_m:
                ga, xra, gla = pend_m.pop(0)
                pend_acc.append((ga, xra, gla, masks(ga)))
            if newgrp is not None:
                gt0, xc_t, xr_t, g = newgrp
                # hT[j_lo, jc, i] = sum_d W1[d, j] xT[d, i] over GRP tiles
                htp = htpp.tile([P, 2, GRP * P], F32)
                for jc in range(2):
                    for dc in range(2):
                        nc.tensor.matmul(
                            htp[:, jc, :],
                            lhsT=w1sb[:, dc, jc * P : (jc + 1) * P],
                            rhs=xc_t[:, dc, g * GRP * P : (g + 1) * GRP * P],
                            start=(dc == 0),
                            stop=(dc == 1),
                        )
            sc = None
            if pend:
                p_gt0, p_htp, p_xr, p_g = pend.pop(0)
                sc = (p_gt0, tanh_act(p_gt0, p_htp))
                pend_m.append((p_gt0, p_xr, p_g))
            scores_acc(sc, ac)
            if newgrp is not None:
                pend.append((gt0, htp, xr_t, g))

        for c in range(C):
            xr = xrp.tile([P, CH, D], BF16)
            nc.sync.dma_start(xr[:], xrow[:, c, :].rearrange("p (t d) -> p t d", t=CH))
            xc = xcp.tile([P, 2, CH * P], BF16)
            nc.sync.dma_start(
                xc[:], xcol[:, c, :].rearrange("p (dc i) -> p dc i", dc=2)
            )
            for g in range(CH // GRP):
                step((c * CH + g * GRP, xc, xr, g))
        while pend or pend_m or pend_acc:
            step(None)

        tmpA = outp.tile([P, D], F32, tag="tmpA")
        nc.vector.tensor_copy(tmpA[:], accs[0][:, 0:D])
        out_sb = outp.tile([P, D], F32, tag="out_sb")
        nc.vector.scalar_tensor_tensor(
            out_sb[:], accs[1][:, 0:D], 0.0, tmpA[:],
            op0=mybir.AluOpType.add, op1=mybir.AluOpType.add,
        )
        nc.sync.dma_start(pooled[:], out_sb[:])
        nc.sync.dma_start(evec_out[:], evec[:])

    nc.compile()
    return nc


def _prep_inputs(x, batch, W1, b1, W2):
    """Shard rows at graph boundaries; pad to a common multiple of CH*P rows.
    Pre-pack both bf16 layouts of x per core."""
    x = np.asarray(x)
    batch = np.asarray(batch)
    bounds = np.searchsorted(batch, np.arange(0, NUM_GRAPHS + 1, GPC))
    counts = np.diff(bounds)
    chunk = CH * P
    R = int(np.ceil(max(int(counts.max()), 1) / chunk) * chunk)
    T = R // P
    C = T // CH

    w1h = np.ascontiguousarray(
        np.asarray(W1, dtype=np.float32).reshape(2, P, D).transpose(1, 0, 2)
    ).astype(ml_dtypes.bfloat16)  # [d_lo, dc, j]
    w2c = np.asarray(W2, dtype=np.float32).reshape(2, P).transpose(1, 0)  # [j_lo, jc]
    w2h = np.ascontiguousarray(
        np.repeat(w2c[:, :, None], 2, axis=2)
    ).astype(ml_dtypes.bfloat16)  # [j_lo, jc, dup]
    b1h = np.asarray(b1, dtype=np.float32)
    with_b1 = bool(np.any(b1h))
    b1p = np.ascontiguousarray(b1h.reshape(2, P).transpose(1, 0))  # [j_lo, jc]

    in_maps = []
    for c in range(NC):
        lo, hi = int(bounds[c]), int(bounds[c + 1])
        n = hi - lo
        xs = np.zeros((R, D), dtype=ml_dtypes.bfloat16)
        xs[:n] = x[lo:hi].astype(ml_dtypes.bfloat16)
        # xrow[p, c, t*D+d] = xs[c*2048 + t*128 + p, d]
        xrow = np.ascontiguousarray(
            xs.reshape(C, CH, P, D).transpose(2, 0, 1, 3).reshape(P, C, CH * D)
        )
        # xcol[dl, c, dc*2048+i] = xs[c*2048 + i, dc*128 + dl]
        xcol = np.ascontiguousarray(
            xs.reshape(C, CH * P, 2, P).transpose(3, 0, 2, 1).reshape(P, C, 2 * CH * P)
        )
        br = np.full((R,), -1.0, dtype=np.float32)
        br[:n] = (np.asarray(batch[lo:hi], dtype=np.int64) - c * GPC).astype(
            np.float32
        )
        brel_pt = np.ascontiguousarray(br.reshape(T, P).transpose(1, 0))  # [P, T]
        m = {"xrow": xrow, "xcol": xcol, "brel": brel_pt, "w1": w1h, "w2": w2h}
        if with_b1:
            m["b1d"] = b1p
        in_maps.append(m)
    return in_maps, R, T, with_b1, [int(c) for c in counts]


def run(x, batch, W1, b1, W2, b2, trace=False, trace_kwargs=None):
    in_maps, R, T, with_b1, counts = _prep_inputs(x, batch, W1, b1, W2)
    nc = build_program(R, T, with_b1)
    res = run_bass_kernel_spmd(
        nc,
        in_maps,
        core_ids=list(range(NC)),
        trace=trace,
        **(trace_kwargs or {}),
    )
    A = np.concatenate(
        [res.results[c]["pooled"] for c in range(NC)], axis=0
    ).astype(np.float64)
    Z = 0.0
    for c in range(NC):
        ev = res.results[c]["evec_out"].astype(np.float64)  # [P, T]
        n = counts[c]
        rows = ev.transpose(1, 0).reshape(-1)  # row r = t*128+p order
        Z += rows[:n].sum()
    out = (A / Z).astype(np.float32)
    return out, res


def kernel(x, batch, W1, b1, W2, b2):
    out, _ = run(x, batch, W1, b1, W2, b2)
    return out
